# revision 28
# baseline (speedup 1.0000x reference)
"""Trainium2 Bass kernel for nn_MultiHeadClassifier.

  logits[b, c] = sum_{(g,l): label_ids[g,l]==c} group_probs[b,g] *
                 (features[b] @ W[g,l] + b[g,l])

Data-parallel over batch (8 cores, 4096 rows each). Per core:
  * Host prep: sort the G*L=1024 head outputs by target class and
    first-fit-pack them into exactly 8 chunks of 128 rows (no padding);
    chunk class bands are near-disjoint — the few overlapped columns are
    handled with accumulate (start=False) scatter matmuls. Host also
    pre-expands group_probs to the packed [1024, BC] layout (pure
    replication) and pre-transposes/casts inputs to fp16.
  * GEMM1 (PE, fp16): glT[gl, b] = W^T.T @ X^T per (chunk, b-tile),
    4 accumulating K=128 matmuls.
  * Fused (DVE): wtj = (pg + bias_j) * ptx_j via scalar_tensor_tensor,
    PSUM in, fp16 SBUF out.
  * Scatter (PE, fp16): logits[b, lo:hi] = wtj^T @ S_j per 128-row
    b-slice; bands cover [0,C) contiguously, overlaps accumulate.
  * Drain (ACT): PSUM -> fp16 SBUF; out-DMA on the sync queue.
Output is fp16 on device, cast to fp32 on host.
"""
import os
import sys
import numpy as np

for _p in ("/opt/trn_rl_repo",):
    if _p not in sys.path:
        sys.path.append(_p)

import concourse.bass as bass  # noqa: E402
import concourse.tile as tile  # noqa: E402
from concourse import bacc, mybir, bass_utils  # noqa: E402
from contextlib import ExitStack  # noqa: E402

F32 = mybir.dt.float32
F16 = mybir.dt.float16

B, F, G, L, C = 32768, 512, 16, 64, 1000
NCORE = 8
BC = B // NCORE          # 4096 batch rows per core
NT = BC // 512           # 8 b-tiles of 512
KF = F // 128            # 4 feature chunks
NCH = 8                  # 8 chunks of 128 head-outputs (exact, no pad)

LAST_EXEC_NS = None


def _host_prep(W, b, label_ids):
    """Pack the GL=1024 (group,label) rows into 8 chunks of exactly 128,
    classes kept whole per chunk when possible (first-fit in sorted class
    order; splits a class only if nothing fits). Returns packed W/bias/S
    plus the scatter segment list."""
    lab = np.asarray(label_ids).reshape(-1).astype(np.int64)
    GL = lab.shape[0]
    Wflat = np.asarray(W, dtype=np.float32).reshape(GL, F)
    bflat = np.asarray(b, dtype=np.float32).reshape(GL)

    order = np.argsort(lab, kind="stable")
    classes, starts = np.unique(lab[order], return_index=True)
    starts = list(starts) + [GL]
    # queue of [class, list of gl rows]
    queue = [[int(classes[i]), list(order[starts[i]:starts[i + 1]])]
             for i in range(len(classes))]

    chunks = []          # list of list[(gl, class)]
    cur, cap = [], 128
    while queue:
        placed = False
        for qi in range(len(queue)):
            c, rows = queue[qi]
            if len(rows) <= cap:
                cur += [(gl, c) for gl in rows]
                cap -= len(rows)
                queue.pop(qi)
                placed = True
                break
        if not placed:
            c, rows = queue[0]           # split the front class
            cur += [(gl, c) for gl in rows[:cap]]
            queue[0][1] = rows[cap:]
            cap = 0
        if cap == 0:
            chunks.append(sorted(cur, key=lambda x: x[1]))
            cur, cap = [], 128
    assert not cur and len(chunks) == NCH

    chunks.sort(key=lambda ch: ch[0][1])
    # class bands and contiguous cover blocks
    blocks = []          # (blk_lo, blk_hi) per chunk, S columns span this
    segments = []        # (j, c0, c1, s_ofs, accum)
    cov = 0
    s_off = []
    off = 0
    for j, ch in enumerate(chunks):
        lo = ch[0][1]
        hi = ch[-1][1] + 1
        blk_lo = min(lo, cov)
        blk_hi = max(hi, cov)
        if j == NCH - 1:
            blk_hi = max(blk_hi, C)
        if j == 0:
            blk_lo = 0
        blocks.append((blk_lo, blk_hi))
        s_off.append(off)
        # accumulate part: columns already covered
        if blk_lo < cov:
            segments.append((j, blk_lo, min(cov, blk_hi), off, True))
        # fresh part
        if blk_hi > cov:
            segments.append((j, max(blk_lo, cov), blk_hi,
                             off + max(blk_lo, cov) - blk_lo, False))
        cov = max(cov, blk_hi)
        off += blk_hi - blk_lo
    assert cov == C, f"cover ended at {cov}"
    SSW = off

    # split segments at 512-column PSUM bank boundaries
    segs = []
    for (j, c0, c1, s0, acc) in segments:
        while c0 < c1:
            nxt = min(c1, (c0 // 512 + 1) * 512)
            segs.append((j, c0, nxt, s0, acc))
            s0 += nxt - c0
            c0 = nxt

    WT = np.zeros((F, NCH * 128), dtype=np.float16)
    biasT = np.zeros((128, NCH), dtype=np.float32)
    SS = np.zeros((128, SSW), dtype=np.float16)
    gmap = np.zeros((NCH, 128), dtype=np.int64)
    for j, ch in enumerate(chunks):
        blk_lo, _ = blocks[j]
        for r, (gl, c) in enumerate(ch):
            WT[:, j * 128 + r] = Wflat[gl]
            biasT[r, j] = bflat[gl]
            SS[r, s_off[j] + c - blk_lo] = 1.0
            gmap[j, r] = gl // L
    return dict(WT=WT, biasT=biasT, SS=SS, SSW=SSW, gmap=gmap, segs=segs)


def _build_program(SSW, segs):
    nc = bacc.Bacc("TRN2", target_bir_lowering=False, debug=False,
                   num_devices=NCORE)
    # xk: k-interleaved X^T packed by t-PAIR — row (tp*128+p), col
    # ((t%2)*2048 + k*512 + c) = X^T[k*128+p, t*512+c]. Pair tiles load
    # as one [128, 4096] DMA => 8KB descriptors.
    xk_d = nc.dram_tensor("xk", [(NT // 2) * 128, 2 * KF * 512], F16,
                          kind="ExternalInput").ap()
    # ptx: expanded group probs + 8 bias columns. Layout: chunks 0-3
    # (2048), all 8 bias cols (8), chunks 4-7 (2048) — so the first-half
    # DMA already carries every bias column.
    PW = NCH * 512 + 8
    ptx_d = nc.dram_tensor("ptx", [NT * 128, PW], F16,
                           kind="ExternalInput").ap()
    # wk: k-interleaved W — row p, col (k*1024 + j*128 + m) =
    # W^T[k*128+p, j*128+m]. One [128, 4096] DMA => 8KB descriptors.
    wk_d = nc.dram_tensor("wk", [128, KF * NCH * 128], F16,
                          kind="ExternalInput").ap()
    s_d = nc.dram_tensor("s", [128, SSW], F16, kind="ExternalInput").ap()
    # partition-major output: row p, col ((t*4+bs)*C + c) holds
    # logits[t*512+bs*128+p, c]; host un-permutes. 8KB descriptors.
    out_d = nc.dram_tensor("logits", [128, NT * 4 * C], F16,
                           kind="ExternalOutput").ap()

    with tile.TileContext(nc) as tc, ExitStack() as ctx:
        const = ctx.enter_context(tc.tile_pool(name="const", bufs=1))
        psG = ctx.enter_context(tc.tile_pool(name="psG", bufs=4, space="PSUM"))
        psL = ctx.enter_context(tc.tile_pool(name="psL", bufs=2, space="PSUM"))
        sbW = ctx.enter_context(tc.tile_pool(name="sbW", bufs=18))
        sbO = ctx.enter_context(tc.tile_pool(name="sbO", bufs=2))

        # W: two k-interleaved half tiles on the scalar queue (the GEMM
        # k=0,1 matmuls only gate on the first), interleaved with the
        # first ptx halves so neither stalls the t=0 chain for long
        wtsA = const.tile([128, 2048], F16, name="wtsA", tag="wtsA")
        nc.scalar.dma_start(wtsA[:], wk_d[:, :2048])
        wtsB = const.tile([128, 2048], F16, name="wtsB", tag="wtsB")
        nc.scalar.dma_start(wtsB[:], wk_d[:, 2048:])
        # X^T on sync: singles for t0/t1 (gate the pipeline start), then
        # the scatter matrix, then 8KB-descriptor pair tiles for t2-7
        xtile = [None] * NT     # (tile, col_base) per t
        x0 = const.tile([128, 2048], F16, name="x0", tag="x0")
        nc.sync.dma_start(x0[:], xk_d[0:128, 0:2048])
        xtile[0] = (x0, 0)
        x1 = const.tile([128, 2048], F16, name="x1", tag="x1")
        nc.sync.dma_start(x1[:], xk_d[0:128, 2048:4096])
        xtile[1] = (x1, 0)
        ss = const.tile([128, SSW], F16, name="ss", tag="ss")
        nc.sync.dma_start(ss[:], s_d[:])
        for tp in range(1, NT // 2):
            t_ = const.tile([128, 4096], F16, name=f"xp{tp}", tag=f"xp{tp}")
            nc.sync.dma_start(t_[:], xk_d[tp * 128:(tp + 1) * 128, :])
            xtile[2 * tp] = (t_, 0)
            xtile[2 * tp + 1] = (t_, 2048)
        # expanded group probs (+bias cols): t0-1 on scalar in j-halves
        # (first half carries the bias cols), t2-7 whole on gpsimd
        HB = 2048 + 8            # first-half width incl bias cols
        ptxs = []
        for t in range(NT):
            t_ = const.tile([128, PW], F16, name=f"ptx{t}", tag=f"ptx{t}")
            if t < 2:
                nc.scalar.dma_start(t_[:, :HB],
                                    ptx_d[t * 128:(t + 1) * 128, :HB])
                nc.scalar.dma_start(t_[:, HB:],
                                    ptx_d[t * 128:(t + 1) * 128, HB:])
            else:
                nc.gpsimd.dma_start(t_[:], ptx_d[t * 128:(t + 1) * 128, :])
            ptxs.append(t_)

        for t in range(NT):
            xt_, xb = xtile[t]
            wtjs = []
            for j in range(NCH):
                pg = psG.tile([128, 512], F32, name="pg", tag="pg")
                for k in range(KF):
                    wsrc = wtsA if k < 2 else wtsB
                    kk = k % 2
                    nc.tensor.matmul(pg[:],
                                     wsrc[:, kk * 1024 + j * 128:
                                          kk * 1024 + (j + 1) * 128],
                                     xt_[:, xb + k * 512:xb + (k + 1) * 512],
                                     start=(k == 0), stop=(k == KF - 1))
                pcol = j * 512 if j < 4 else HB + (j - 4) * 512
                wtj = sbW.tile([128, 512], F16, name="wtj", tag="wtj")
                nc.vector.scalar_tensor_tensor(
                    wtj[:], pg[:],
                    ptxs[t][:, 2048 + j:2048 + j + 1],
                    ptxs[t][:, pcol:pcol + 512],
                    op0=mybir.AluOpType.add, op1=mybir.AluOpType.mult)
                wtjs.append(wtj)
            if t < NT - 1:
                ob = sbO.tile([128, 4 * C], F16, name="ob", tag="ob")
                for bs_i in range(4):
                    pl = psL.tile([128, 1024], F32, name="pl", tag="pl")
                    for (j, c0, c1, s0, acc) in segs:
                        nc.tensor.matmul(pl[:, c0:c1],
                                         wtjs[j][:, bass.ts(bs_i, 128)],
                                         ss[:, s0:s0 + (c1 - c0)],
                                         start=not acc, stop=True)
                    nc.scalar.activation(ob[:, bs_i * C:(bs_i + 1) * C],
                                         pl[:, :C],
                                         mybir.ActivationFunctionType.Copy,
                                         bias=0.0, scale=1.0)
                nc.scalar.dma_start(out_d[:, t * 4 * C:(t + 1) * 4 * C],
                                    ob[:])
            else:
                # tail: per-bs drains split across both engines, per-bs
                # DMAs split across both queues, to shorten the flush
                for bs_i in range(4):
                    pl = psL.tile([128, 1024], F32, name="pl", tag="pl")
                    for (j, c0, c1, s0, acc) in segs:
                        nc.tensor.matmul(pl[:, c0:c1],
                                         wtjs[j][:, bass.ts(bs_i, 128)],
                                         ss[:, s0:s0 + (c1 - c0)],
                                         start=not acc, stop=True)
                    obs = sbO.tile([128, C], F16, name="obs", tag="obs")
                    nc.scalar.activation(obs[:, :512], pl[:, :512],
                                         mybir.ActivationFunctionType.Copy,
                                         bias=0.0, scale=1.0)
                    nc.vector.tensor_copy(obs[:, 512:], pl[:, 512:C])
                    oeng = nc.sync if bs_i % 2 == 0 else nc.scalar
                    oeng.dma_start(
                        out_d[:, (t * 4 + bs_i) * C:(t * 4 + bs_i + 1) * C],
                        obs[:])
    nc.finalize()
    return nc


def kernel(features, group_probs, W, b, label_ids):
    global LAST_EXEC_NS
    features = np.asarray(features, dtype=np.float32)
    group_probs = np.asarray(group_probs, dtype=np.float32)
    prep = _host_prep(W, b, label_ids)
    nc = _build_program(prep["SSW"], prep["segs"])

    XT = features.T.astype(np.float16)                        # [F, B]
    PT = group_probs.T.astype(np.float16)                     # [G, B]
    gmap = prep["gmap"]
    # k-interleaved W: [128, KF*NCH*128]
    WTf = prep["WT"]                                          # [F, 1024]
    wk = np.empty((128, KF * NCH * 128), dtype=np.float16)
    for k in range(KF):
        wk[:, k * 1024:(k + 1) * 1024] = WTf[k * 128:(k + 1) * 128, :]
    bias16 = prep["biasT"].astype(np.float16)                 # [128, NCH]
    in_maps = []
    for c in range(NCORE):
        # k-interleaved X^T packed by t-pair:
        # xk[tp*128+p, (t%2)*2048 + k*512+cc] = XT[k*128+p, t*512+cc]
        xc = XT[:, c * BC:(c + 1) * BC].reshape(KF, 128, NT // 2, 2, 512)
        xk = np.ascontiguousarray(
            xc.transpose(2, 1, 3, 0, 4).reshape((NT // 2) * 128,
                                                2 * KF * 512))
        ptc = PT[:, c * BC:(c + 1) * BC].reshape(G, NT, 512)  # [16, 8, 512]
        ptx = np.empty((NT, 128, NCH * 512 + 8), dtype=np.float16)
        for j in range(NCH):
            pcol = j * 512 if j < 4 else 2056 + (j - 4) * 512
            # [128, NT, 512] -> [NT, 128, 512]
            ptx[:, :, pcol:pcol + 512] = ptc[gmap[j]].transpose(1, 0, 2)
        ptx[:, :, 2048:2056] = bias16[None, :, :]
        in_maps.append({
            "xk": xk,
            "ptx": np.ascontiguousarray(ptx.reshape(NT * 128, -1)),
            "wk": wk,
            "s": prep["SS"],
        })

    trace = bool(os.environ.get("BASS_TRACE"))
    if trace:
        bass_utils.upload_artifacts = lambda d: "local://skipped"
    try:
        res = bass_utils.run_bass_kernel_spmd(nc, in_maps,
                                              core_ids=list(range(NCORE)))
    except Exception:
        # transient NRT device errors have been observed; one retry
        res = bass_utils.run_bass_kernel_spmd(nc, in_maps,
                                              core_ids=list(range(NCORE)))
    if trace:
        LAST_EXEC_NS = res.exec_time_ns
        if res.exec_time_ns is not None:
            print(f"HW exec time: {res.exec_time_ns} ns")

    parts = []
    for c in range(NCORE):
        o2 = res.results[c]["logits"].reshape(128, NT * 4, C)
        # [p, t*4+bs, c] -> [t*4+bs, p, c] -> [BC, C]
        parts.append(o2.transpose(1, 0, 2).reshape(BC, C))
    out = np.concatenate(parts, axis=0)
    return np.ascontiguousarray(out.astype(np.float32))
